# revision 14
# baseline (speedup 1.0000x reference)
"""BitwiseMLP Trainium2 kernel: 8-way data-parallel over the batch dim.

Math (per reference):
  h0 = x @ W0.T + b0; h0 = BN0(h0); s0 = sign(h0)
  h1 = s0 @ sign(W1).T + b1; h1 = BN1(h1); s1 = sign(h1)
  out = (s1 @ sign(W2).T + b2) * out_scale

Device strategy (per core, batch shard of 8192 rows; activations stay
transposed [channel, batch] end-to-end so the device does zero transposes):
  - L0 = x@W0.T to ~2^-16.5 accuracy via a main pass + one fp8 chain:
      xh = fp16(x*2^10), W11 = fp16(W0*2^18) (11-bit RNE splits);
      main: xh @ W11 as fp16, exact, products at scale 2^28;
      corrections: xl@W0 + x@Wr  (xl = x - xh/2^10, Wr = W0 - W11/2^18)
      as ONE fp8e4 DoubleRow chain of K=2048: slots
      [e4m3(xl*2^16) @ e4m3(W0*2^12)] and [e4m3(x*2^5) @ e4m3(Wr*2^23)].
      Every product lands at scale 2^28, so all 16 MMs accumulate into
      one PSUM; the 2^-28 folds into the BN scale of the Sign activation.
  - BN+sign fuse into one ScalarE activation per tile:
    s = Sign(psum * A + B) with per-channel A/B, output fp8e4 (+-1 exact).
  - L1/L2 are exact +-1 fp8e4 matmuls with DoubleRow (2x rate); results are
    small even integers accumulated exactly in fp32 PSUM.
  - Final eviction: Identity activation out = psum*out_scale + b2*out_scale.
  - The PE runs at the 216ns/MM streaming floor (3072 MMs = 663us).
    Startup is DMA-bound AND issue-order-bound: chains run corrections
    (small fp8 operands) before mains, DMAs are issued in consumption
    order on the sync DGE queue (w0h AFTER the correction stream so it
    does not steal HBM bandwidth from the critical path); w1/w2 (needed
    ~40us in) load from the scalar DGE queue.  15 HAM warm-up MMs cover
    the boot window so real chains run at 2.4 GHz.
  - All layouts keep DMA runs contiguous per partition (2-16 KB) and every
    DoubleRow weight pair a contiguous 256 B SBUF read; SBUF tile placement
    (pool order) is perf-critical (bank-phase contention costs +20%).
Host does the batch shard, the transposes and the hi/lo splits; the output
comes back transposed per core and is re-assembled in numpy.
"""
import os
import sys
import types

import numpy as np
import ml_dtypes

import concourse.bass as bass
import concourse.mybir as mybir
import concourse.tile as tile
from concourse import bacc
from concourse.bass_utils import run_bass_kernel_spmd


def _ensure_axon_hooks():
    """concourse.bass_utils imports antenv.axon_hooks when tracing is
    requested (BASS_TRACE=1). The trimmed image lacks that module, which
    would turn an optional profile into a crash — synthesize it, wiring the
    real NTFF hook when libaxon_pjrt.so is present."""
    try:
        import antenv.axon_hooks  # noqa: F401
        return
    except ImportError:
        pass
    try:
        import antenv
    except ImportError:
        return
    mod = types.ModuleType("antenv.axon_hooks")
    state = {"hook": None}
    mod.set_axon_ntff_profile_hook = lambda h: state.update(hook=h)
    mod.get_axon_ntff_profile_hook = lambda: state["hook"]
    sys.modules["antenv.axon_hooks"] = mod
    antenv.axon_hooks = mod
    so = "/opt/axon/libaxon_pjrt.so"
    if os.path.exists(so):
        try:
            from trn_agent_boot.trn_boot import _ntff_profile_via_ctypes
            mod.set_axon_ntff_profile_hook(_ntff_profile_via_ctypes(so))
            import concourse.bass_utils as _bu
            _real_upload = _bu.upload_artifacts

            def _safe_upload(tmpdir):
                try:
                    return _real_upload(tmpdir)
                except Exception:
                    return f"local:{tmpdir}"

            _bu.upload_artifacts = _safe_upload
        except Exception:
            pass


_ensure_axon_hooks()

dt = mybir.dt
P = 128
D = 1024
B = 65536
NCORES = 8
BS = B // NCORES          # 8192 batch rows per core
BT = 512                  # batch-tile width (columns of transposed activations)
NBT = BS // BT            # 16 batch tiles per core
KO = D // P               # 8 k-subtiles of 128 channels
NQ = 2 * KO               # fp8 correction slots: [xl*2^17 (8) | x*2^5 (8)]
XQB = NQ * BT             # 8192 bytes of fp8 correction slots per tile
XHB = KO * BT * 2         # 8192 bytes of fp16 main slots per tile
XINB = XQB + XHB          # 16384 bytes per partition per batch-tile
EPS = 1e-5
PSC = np.float32(2.0**28)  # uniform product scale of the L0 PSUM
XSC = np.float32(2.0**10)  # fp16 main moving-operand scale
WSC = np.float32(2.0**18)  # fp16 main weight scale (XSC*WSC = PSC)

LAST_RESULTS = None       # BassKernelResults of the most recent run (for profiling)
_NC = None                # cached compiled Bass module (build once per process)


def _build():
    # Layouts are chosen so every DMA chunk is contiguous per partition
    # (2-16 KB runs) and every DoubleRow weight pair a contiguous 256 B
    # SBUF read: weights are m-block-major [P, M, K, 128], activations are
    # batch-tile-major [P, NBT, XINB] packed bytes (fp8 slots then fp16).
    nc = bacc.Bacc(num_devices=NCORES)
    xin = nc.dram_tensor("xin", [P, NBT, XINB], dt.uint8, kind="ExternalInput")
    w0h = nc.dram_tensor("w0h", [P, KO, KO, P], dt.float16, kind="ExternalInput")
    wq = nc.dram_tensor("wq", [P, KO, NQ, P], dt.float8e4, kind="ExternalInput")
    w1 = nc.dram_tensor("w1", [P, KO, KO, P], dt.float8e4, kind="ExternalInput")
    w2 = nc.dram_tensor("w2", [P, KO, KO, P], dt.float8e4, kind="ExternalInput")
    vec = nc.dram_tensor("vec", [P, 6, KO], dt.float32, kind="ExternalInput")
    out = nc.dram_tensor("out", [P, NBT, KO, BT], dt.float32, kind="ExternalOutput")

    Sign = mybir.ActivationFunctionType.Sign
    Ident = mybir.ActivationFunctionType.Identity
    DR = mybir.MatmulPerfMode.DoubleRow

    def xq_dram(bt, a, b):
        # fp8 slots a..b of batch-tile bt in the packed dram tensor
        return xin[:, bt, a * BT:b * BT].bitcast(dt.float8e4)

    def xh_dram(bt):
        # fp16 main half of batch-tile bt
        return xin[:, bt, XQB:].bitcast(dt.float16)

    with tile.TileContext(nc) as tc:
        with (
            tc.tile_pool(name="wpool", bufs=1) as wpool,
            tc.tile_pool(name="xpool", bufs=2) as xpool,
            tc.tile_pool(name="spool", bufs=2) as spool,
            tc.tile_pool(name="opool", bufs=3) as opool,
            tc.tile_pool(name="pspool", bufs=8, space="PSUM") as pspool,
        ):
            w0h_sb = wpool.tile([P, KO, KO, P], dt.float16)
            wq_sb = wpool.tile([P, KO, NQ, P], dt.float8e4)
            w1_sb = wpool.tile([P, KO, KO, P], dt.float8e4)
            w2_sb = wpool.tile([P, KO, KO, P], dt.float8e4)
            vec_sb = wpool.tile([P, 6, KO], dt.float32)

            w0h_t, wq_t = w0h[:], wq[:]
            out_t = out[:]

            # Startup is DMA-bound, so chains run corrections (small fp8
            # operands) before mains (bigger fp16 operands), DMAs are issued
            # in consumption order in per-m-block chunks, and bt0 runs ALL
            # its correction chains first; w0h stays BEHIND the correction
            # stream on the sync queue so it cannot steal HBM bandwidth,
            # while w1/w2 (needed much later) ride the scalar DGE queue.
            xh_sb0 = xpool.tile([P, KO, BT], dt.float16, tag="xh")
            xq_sb0 = xpool.tile([P, NQ, BT], dt.float8e4, tag="xq")
            # HAM warm-up: dependency-free dummy MMs keep the PE busy from
            # ~2us until the first real operands land, so the first
            # real chains run at 2.4 GHz instead of the cold 1.2 GHz.  The
            # first real start=True resets the bank; results never read.
            warm_sb = wpool.tile([P, P + BT], dt.float16)
            nc.vector.memset(warm_sb, 0.0)
            ps_w = pspool.tile([P, BT], dt.float32, tag="ps", name="ps_warm")
            for i in range(20):
                nc.tensor.matmul(ps_w, warm_sb[:, 0:P], warm_sb[:, P:P + BT],
                                 start=i == 0, stop=i == 19)
            nc.sync.dma_start(vec_sb, vec[:])
            nc.sync.dma_start(wq_sb[:, 0], wq_t[:, 0])
            for c in range(4):
                nc.sync.dma_start(xq_sb0[:, 4 * c:4 * c + 4, :],
                                  xq_dram(0, 4 * c, 4 * c + 4))
            for m in range(1, KO):
                nc.sync.dma_start(wq_sb[:, m], wq_t[:, m])
            nc.sync.dma_start(xh_sb0, xh_dram(0))
            for m in range(KO):
                nc.scalar.dma_start(w0h_sb[:, m], w0h_t[:, m])
            nc.scalar.dma_start(w1_sb, w1[:])
            nc.scalar.dma_start(w2_sb, w2[:])

            for bt in range(NBT):
                if bt == 0:
                    xh_sb, xq_sb = xh_sb0, xq_sb0
                else:
                    xh_sb = xpool.tile([P, KO, BT], dt.float16, tag="xh")
                    xq_sb = xpool.tile([P, NQ, BT], dt.float8e4, tag="xq")
                    nc.sync.dma_start(xq_sb, xq_dram(bt, 0, NQ))
                    nc.sync.dma_start(xh_sb, xh_dram(bt))

                # ---- L0: fp8 DR corrections + fp16 main, fused BN0+sign ----
                s0_sb = spool.tile([P, KO, BT], dt.float8e4, tag="s0")
                if bt == 0:
                    # startup special case: all correction chains first (they
                    # only need the small fp8 tensors); main DMAs land behind.
                    ps_l = [pspool.tile([P, BT], dt.float32, tag="ps",
                                        name=f"ps0_{i}")
                            for i in range(KO)]
                    for m in range(KO):
                        for i in range(KO):
                            nc.tensor.matmul(ps_l[m],
                                             wq_sb[:, m, 2 * i:2 * i + 2, :],
                                             xq_sb[:, 2 * i:2 * i + 2, :],
                                             start=i == 0, stop=False,
                                             perf_mode=DR)
                    for m in range(KO):
                        for k in range(KO):
                            nc.tensor.matmul(ps_l[m], w0h_sb[:, m, k, :],
                                             xh_sb[:, k, :],
                                             start=False, stop=k == KO - 1)
                        nc.scalar.activation(s0_sb[:, m, :], ps_l[m], Sign,
                                             bias=vec_sb[:, 1, m:m + 1],
                                             scale=vec_sb[:, 0, m:m + 1])
                else:
                    for m in range(KO):
                        ps = pspool.tile([P, BT], dt.float32, tag="ps")
                        for i in range(KO):
                            nc.tensor.matmul(ps,
                                             wq_sb[:, m, 2 * i:2 * i + 2, :],
                                             xq_sb[:, 2 * i:2 * i + 2, :],
                                             start=i == 0, stop=False,
                                             perf_mode=DR)
                        for k in range(KO):
                            nc.tensor.matmul(ps, w0h_sb[:, m, k, :],
                                             xh_sb[:, k, :],
                                             start=False, stop=k == KO - 1)
                        nc.scalar.activation(s0_sb[:, m, :], ps, Sign,
                                             bias=vec_sb[:, 1, m:m + 1],
                                             scale=vec_sb[:, 0, m:m + 1])

                # ---- L1: fp8 +-1 DoubleRow matmuls, fused BN1+sign ----
                s1_sb = spool.tile([P, KO, BT], dt.float8e4, tag="s1")
                for m in range(KO):
                    ps = pspool.tile([P, BT], dt.float32, tag="ps")
                    for kp in range(KO // 2):
                        nc.tensor.matmul(ps, w1_sb[:, m, 2 * kp:2 * kp + 2, :],
                                         s0_sb[:, 2 * kp:2 * kp + 2, :],
                                         start=kp == 0, stop=kp == KO // 2 - 1,
                                         perf_mode=DR)
                    nc.scalar.activation(s1_sb[:, m, :], ps, Sign,
                                         bias=vec_sb[:, 3, m:m + 1],
                                         scale=vec_sb[:, 2, m:m + 1])

                # ---- L2: fp8 +-1 DoubleRow matmuls, fused scale+bias ----
                # The last tile evicts 2/2/2/1/1 through a DEDICATED 5-deep
                # buffer tag: no opool wrap-waits delay the final acts, and
                # the final transfer (the kernel tail) is only 256KB.
                if bt < NBT - 1:
                    groups = [(0, 4), (4, 4)]
                else:
                    groups = [(0, 2), (2, 2), (4, 2), (6, 1), (7, 1)]
                for m0, ng in groups:
                    if bt < NBT - 1:
                        o_sb = opool.tile([P, ng, BT], dt.float32, tag="om")
                    else:
                        o_sb = opool.tile([P, ng, BT], dt.float32, tag="omz",
                                          bufs=5)
                    for mi in range(ng):
                        m = m0 + mi
                        ps = pspool.tile([P, BT], dt.float32, tag="ps")
                        for kp in range(KO // 2):
                            nc.tensor.matmul(ps, w2_sb[:, m, 2 * kp:2 * kp + 2, :],
                                             s1_sb[:, 2 * kp:2 * kp + 2, :],
                                             start=kp == 0, stop=kp == KO // 2 - 1,
                                             perf_mode=DR)
                        nc.scalar.activation(o_sb[:, mi, :], ps, Ident,
                                             bias=vec_sb[:, 5, m:m + 1],
                                             scale=vec_sb[:, 4, m:m + 1])
                    nc.sync.dma_start(out_t[:, bt, m0:m0 + ng, :], o_sb)

    nc.compile()
    return nc


def kernel(**inputs) -> np.ndarray:
    global LAST_RESULTS
    f32 = np.float32
    x = np.asarray(inputs["x"], f32)
    W0 = np.asarray(inputs["W0"], f32)
    b0 = np.asarray(inputs["b0"], f32)
    W1 = np.asarray(inputs["W1"], f32)
    b1 = np.asarray(inputs["b1"], f32)
    W2 = np.asarray(inputs["W2"], f32)
    b2 = np.asarray(inputs["b2"], f32)
    bn0_g = np.asarray(inputs["bn0_g"], f32)
    bn0_b = np.asarray(inputs["bn0_b"], f32)
    bn0_rm = np.asarray(inputs["bn0_rm"], f32)
    bn0_rv = np.asarray(inputs["bn0_rv"], f32)
    bn1_g = np.asarray(inputs["bn1_g"], f32)
    bn1_b = np.asarray(inputs["bn1_b"], f32)
    bn1_rm = np.asarray(inputs["bn1_rm"], f32)
    bn1_rv = np.asarray(inputs["bn1_rv"], f32)
    osc = np.asarray(inputs["out_scale"], f32)

    # per-channel affine folds (BN in eval mode):
    #   bn0(h+b0) = h*A0 + B0 ; bn1(h+b1) = h*A1 + B1 ; out = h*CS + CB
    # L0's psum carries h*2^28, so A0 absorbs the 2^-28.
    inv0 = (bn0_g / np.sqrt(bn0_rv + EPS)).astype(f32)
    inv1 = (bn1_g / np.sqrt(bn1_rv + EPS)).astype(f32)
    A0, B0 = (inv0 / PSC).astype(f32), ((b0 - bn0_rm) * inv0 + bn0_b).astype(f32)
    A1, B1 = inv1, ((b1 - bn1_rm) * inv1 + bn1_b).astype(f32)
    CS, CB = osc, (b2 * osc).astype(f32)
    vec = np.stack([A0, B0, A1, B1, CS, CB])           # [6, D]
    vec_host = np.ascontiguousarray(
        vec.reshape(6, KO, P).transpose(2, 0, 1))      # [P, 6, KO]

    def wfmt(a):
        # [D_out, D_in] -> [P(p), KO(m), KO(k), P(c)] with
        # elem[p, m, k, c] = a[m*128 + c, k*128 + p]
        return a.reshape(KO, P, KO, P).transpose(3, 0, 2, 1)

    def xfmt(a):
        # [B, D] -> [P(p), B/BT(bt), KO(k), BT(c)] with
        # elem[p, bt, k, c] = a[bt*BT + c, k*128 + p]
        return a.reshape(-1, BT, KO, P).transpose(3, 0, 2, 1)

    e4 = mybir.dt.np(dt.float8e4)
    W11s = np.float16(W0 * WSC)                        # fp16 main weights
    Wr = W0 - W11s.astype(f32) / WSC
    w0h_host = np.ascontiguousarray(wfmt(W11s))
    wq_host = np.ascontiguousarray(np.concatenate([
        wfmt((W0 * np.float32(2.0**12)).astype(e4)),
        wfmt((Wr * np.float32(2.0**23)).astype(e4)),
    ], axis=2))                                        # [P, KO, 2KO, P]
    w1_host = np.ascontiguousarray(wfmt(np.sign(W1).astype(e4)))
    w2_host = np.ascontiguousarray(wfmt(np.sign(W2).astype(e4)))

    xh11s = np.float16(x * XSC)                        # fp16 main moving
    xl = x - xh11s.astype(f32) / XSC
    xhT = xfmt(xh11s)                                  # [P, B/BT, KO, BT]
    xqT = np.concatenate([
        xfmt((xl * np.float32(2.0**16)).astype(e4)),
        xfmt((x * np.float32(2.0**5)).astype(e4)),
    ], axis=2)                                         # [P, B/BT, 2KO, BT]
    # pack fp8 slots + fp16 main bytes into one dram tensor per batch-tile
    xq_bytes = np.ascontiguousarray(xqT).view(np.uint8).reshape(P, B // BT, XQB)
    xh_bytes = np.ascontiguousarray(xhT).view(np.uint8).reshape(P, B // BT, XHB)

    shared = {
        "w0h": w0h_host, "wq": wq_host,
        "w1": w1_host, "w2": w2_host, "vec": vec_host,
    }
    in_maps = []
    for c in range(NCORES):
        bs = slice(c * NBT, (c + 1) * NBT)
        xin_host = np.concatenate(
            [xq_bytes[:, bs], xh_bytes[:, bs]], axis=2)
        in_maps.append({
            **shared,
            "xin": np.ascontiguousarray(xin_host),
        })

    global _NC
    if _NC is None:
        _NC = _build()
    res = run_bass_kernel_spmd(_NC, in_maps, core_ids=list(range(NCORES)))
    LAST_RESULTS = res

    out = np.empty((B, D), f32)
    for c in range(NCORES):
        # [P, NBT, KO, BT] -> [BS, KO*P] with channel = ko*P + p
        o = res.results[c]["out"].transpose(1, 3, 2, 0).reshape(BS, D)
        out[c * BS:(c + 1) * BS] = o
    return out


# revision 15
# speedup vs baseline: 1.2106x; 1.2106x over previous
"""BitwiseMLP Trainium2 kernel: 8-way data-parallel over the batch dim.

Math (per reference):
  h0 = x @ W0.T + b0; h0 = BN0(h0); s0 = sign(h0)
  h1 = s0 @ sign(W1).T + b1; h1 = BN1(h1); s1 = sign(h1)
  out = (s1 @ sign(W2).T + b2) * out_scale

Device strategy (per core, batch shard of 8192 rows; activations stay
transposed [channel, batch] end-to-end so the device does zero transposes):
  - L0 = x@W0.T to ~2^-16.5 accuracy via a main pass + one fp8 chain:
      xh = fp16(x*2^10), W11 = fp16(W0*2^18) (11-bit RNE splits);
      main: xh @ W11 as fp16, exact, products at scale 2^28;
      corrections: xl@W0 + x@Wr  (xl = x - xh/2^10, Wr = W0 - W11/2^18)
      as ONE fp8e4 DoubleRow chain of K=2048: slots
      [e4m3(xl*2^16) @ e4m3(W0*2^12)] and [e4m3(x*2^5) @ e4m3(Wr*2^23)].
      Every product lands at scale 2^28, so all 16 MMs accumulate into
      one PSUM; the 2^-28 folds into the BN scale of the Sign activation.
  - BN+sign fuse into one ScalarE activation per tile:
    s = Sign(psum * A + B) with per-channel A/B, output fp8e4 (+-1 exact).
  - L1/L2 are exact +-1 fp8e4 matmuls with DoubleRow (2x rate); results are
    small even integers accumulated exactly in fp32 PSUM.
  - Final eviction: Identity activation out = psum*out_scale + b2*out_scale.
  - The PE runs at the 216ns/MM streaming floor (3072 MMs = 663us).
    Startup is DMA-bound AND issue-order-bound: chains run corrections
    (small fp8 operands) before mains, DMAs are issued in consumption
    order on the sync DGE queue (w0h AFTER the correction stream so it
    does not steal HBM bandwidth from the critical path); w1/w2 (needed
    ~40us in) load from the scalar DGE queue.  15 HAM warm-up MMs cover
    the boot window so real chains run at 2.4 GHz.
  - All layouts keep DMA runs contiguous per partition (2-16 KB) and every
    DoubleRow weight pair a contiguous 256 B SBUF read; SBUF tile placement
    (pool order) is perf-critical (bank-phase contention costs +20%).
Host does the batch shard, the transposes and the hi/lo splits; the output
comes back transposed per core and is re-assembled in numpy.
"""
import os
import sys
import types

import numpy as np
import ml_dtypes

import concourse.bass as bass
import concourse.mybir as mybir
import concourse.tile as tile
from concourse import bacc
from concourse.bass_utils import run_bass_kernel_spmd


def _ensure_axon_hooks():
    """concourse.bass_utils imports antenv.axon_hooks when tracing is
    requested (BASS_TRACE=1). The trimmed image lacks that module, which
    would turn an optional profile into a crash — synthesize it, wiring the
    real NTFF hook when libaxon_pjrt.so is present."""
    try:
        import antenv.axon_hooks  # noqa: F401
        return
    except ImportError:
        pass
    try:
        import antenv
    except ImportError:
        return
    mod = types.ModuleType("antenv.axon_hooks")
    state = {"hook": None}
    mod.set_axon_ntff_profile_hook = lambda h: state.update(hook=h)
    mod.get_axon_ntff_profile_hook = lambda: state["hook"]
    sys.modules["antenv.axon_hooks"] = mod
    antenv.axon_hooks = mod
    so = "/opt/axon/libaxon_pjrt.so"
    if os.path.exists(so):
        try:
            from trn_agent_boot.trn_boot import _ntff_profile_via_ctypes
            mod.set_axon_ntff_profile_hook(_ntff_profile_via_ctypes(so))
            import concourse.bass_utils as _bu
            _real_upload = _bu.upload_artifacts

            def _safe_upload(tmpdir):
                try:
                    return _real_upload(tmpdir)
                except Exception:
                    return f"local:{tmpdir}"

            _bu.upload_artifacts = _safe_upload
        except Exception:
            pass


_ensure_axon_hooks()

dt = mybir.dt
P = 128
D = 1024
B = 65536
NCORES = 8
BS = B // NCORES          # 8192 batch rows per core
BT = 512                  # batch-tile width (columns of transposed activations)
NBT = BS // BT            # 16 batch tiles per core
KO = D // P               # 8 k-subtiles of 128 channels
NQ = 2 * KO               # fp8 correction slots: [xl*2^17 (8) | x*2^5 (8)]
XQB = NQ * BT             # 8192 bytes of fp8 correction slots per tile
XHB = KO * BT * 2         # 8192 bytes of fp16 main slots per tile
XINB = XQB + XHB          # 16384 bytes per partition per batch-tile
EPS = 1e-5
PSC = np.float32(2.0**28)  # uniform product scale of the L0 PSUM
XSC = np.float32(2.0**10)  # fp16 main moving-operand scale
WSC = np.float32(2.0**18)  # fp16 main weight scale (XSC*WSC = PSC)

LAST_RESULTS = None       # BassKernelResults of the most recent run (for profiling)
_NC = None                # cached compiled Bass module (build once per process)


def _build():
    # Layouts are chosen so every DMA chunk is contiguous per partition
    # (2-16 KB runs) and every DoubleRow weight pair a contiguous 256 B
    # SBUF read: weights are m-block-major [P, M, K, 128], activations are
    # batch-tile-major [P, NBT, XINB] packed bytes (fp8 slots then fp16).
    nc = bacc.Bacc(num_devices=NCORES)
    xin = nc.dram_tensor("xin", [P, NBT, XINB], dt.uint8, kind="ExternalInput")
    w0h = nc.dram_tensor("w0h", [P, KO, KO, P], dt.float16, kind="ExternalInput")
    wq = nc.dram_tensor("wq", [P, KO, NQ, P], dt.float8e4, kind="ExternalInput")
    w1 = nc.dram_tensor("w1", [P, KO, KO, P], dt.float8e4, kind="ExternalInput")
    w2 = nc.dram_tensor("w2", [P, KO, KO, P], dt.float8e4, kind="ExternalInput")
    vec = nc.dram_tensor("vec", [P, 6, KO], dt.float32, kind="ExternalInput")
    out = nc.dram_tensor("out", [P, NBT, KO, BT], dt.float32, kind="ExternalOutput")

    Sign = mybir.ActivationFunctionType.Sign
    Ident = mybir.ActivationFunctionType.Identity
    DR = mybir.MatmulPerfMode.DoubleRow

    def xq_dram(bt, a, b):
        # fp8 slots a..b of batch-tile bt in the packed dram tensor
        return xin[:, bt, a * BT:b * BT].bitcast(dt.float8e4)

    def xh_dram(bt):
        # fp16 main half of batch-tile bt
        return xin[:, bt, XQB:].bitcast(dt.float16)

    with tile.TileContext(nc) as tc:
        with (
            tc.tile_pool(name="wpool", bufs=1) as wpool,
            tc.tile_pool(name="xpool", bufs=2) as xpool,
            tc.tile_pool(name="spool", bufs=2) as spool,
            tc.tile_pool(name="opool", bufs=3) as opool,
            tc.tile_pool(name="pspool", bufs=8, space="PSUM") as pspool,
        ):
            w0h_sb = wpool.tile([P, KO, KO, P], dt.float16)
            wq_sb = wpool.tile([P, KO, NQ, P], dt.float8e4)
            w1_sb = wpool.tile([P, KO, KO, P], dt.float8e4)
            w2_sb = wpool.tile([P, KO, KO, P], dt.float8e4)
            vec_sb = wpool.tile([P, 6, KO], dt.float32)

            w0h_t, wq_t = w0h[:], wq[:]
            out_t = out[:]

            # Startup is DMA-bound, so chains run corrections (small fp8
            # operands) before mains (bigger fp16 operands), DMAs are issued
            # in consumption order in per-m-block chunks, and bt0 runs ALL
            # its correction chains first; w0h stays BEHIND the correction
            # stream on the sync queue so it cannot steal HBM bandwidth,
            # while w1/w2 (needed much later) ride the scalar DGE queue.
            xh_sb0 = xpool.tile([P, KO, BT], dt.float16, tag="xh")
            xq_sb0 = xpool.tile([P, NQ, BT], dt.float8e4, tag="xq")
            # HAM warm-up: dependency-free dummy MMs keep the PE busy from
            # ~2us until the first real operands land, so the first
            # real chains run at 2.4 GHz instead of the cold 1.2 GHz.  The
            # first real start=True resets the bank; results never read.
            warm_sb = wpool.tile([P, P + BT], dt.float16)
            nc.vector.memset(warm_sb, 0.0)
            ps_w = pspool.tile([P, BT], dt.float32, tag="ps", name="ps_warm")
            for i in range(20):
                nc.tensor.matmul(ps_w, warm_sb[:, 0:P], warm_sb[:, P:P + BT],
                                 start=i == 0, stop=i == 19)
            nc.sync.dma_start(vec_sb, vec[:])
            nc.sync.dma_start(wq_sb[:, 0], wq_t[:, 0])
            for c in range(4):
                nc.sync.dma_start(xq_sb0[:, 4 * c:4 * c + 4, :],
                                  xq_dram(0, 4 * c, 4 * c + 4))
            for m in range(1, KO):
                nc.sync.dma_start(wq_sb[:, m], wq_t[:, m])
            nc.sync.dma_start(xh_sb0, xh_dram(0))
            for m in range(KO):
                nc.sync.dma_start(w0h_sb[:, m], w0h_t[:, m])
            nc.sync.dma_start(w1_sb, w1[:])
            nc.sync.dma_start(w2_sb, w2[:])

            for bt in range(NBT):
                if bt == 0:
                    xh_sb, xq_sb = xh_sb0, xq_sb0
                else:
                    xh_sb = xpool.tile([P, KO, BT], dt.float16, tag="xh")
                    xq_sb = xpool.tile([P, NQ, BT], dt.float8e4, tag="xq")
                    nc.sync.dma_start(xq_sb, xq_dram(bt, 0, NQ))
                    nc.sync.dma_start(xh_sb, xh_dram(bt))

                # ---- L0: fp8 DR corrections + fp16 main, fused BN0+sign ----
                s0_sb = spool.tile([P, KO, BT], dt.float8e4, tag="s0")
                if bt == 0:
                    # startup special case: all correction chains first (they
                    # only need the small fp8 tensors); main DMAs land behind.
                    ps_l = [pspool.tile([P, BT], dt.float32, tag="ps",
                                        name=f"ps0_{i}")
                            for i in range(KO)]
                    for m in range(KO):
                        for i in range(KO):
                            nc.tensor.matmul(ps_l[m],
                                             wq_sb[:, m, 2 * i:2 * i + 2, :],
                                             xq_sb[:, 2 * i:2 * i + 2, :],
                                             start=i == 0, stop=False,
                                             perf_mode=DR)
                    for m in range(KO):
                        for k in range(KO):
                            nc.tensor.matmul(ps_l[m], w0h_sb[:, m, k, :],
                                             xh_sb[:, k, :],
                                             start=False, stop=k == KO - 1)
                        nc.scalar.activation(s0_sb[:, m, :], ps_l[m], Sign,
                                             bias=vec_sb[:, 1, m:m + 1],
                                             scale=vec_sb[:, 0, m:m + 1])
                else:
                    for m in range(KO):
                        ps = pspool.tile([P, BT], dt.float32, tag="ps")
                        for i in range(KO):
                            nc.tensor.matmul(ps,
                                             wq_sb[:, m, 2 * i:2 * i + 2, :],
                                             xq_sb[:, 2 * i:2 * i + 2, :],
                                             start=i == 0, stop=False,
                                             perf_mode=DR)
                        for k in range(KO):
                            nc.tensor.matmul(ps, w0h_sb[:, m, k, :],
                                             xh_sb[:, k, :],
                                             start=False, stop=k == KO - 1)
                        nc.scalar.activation(s0_sb[:, m, :], ps, Sign,
                                             bias=vec_sb[:, 1, m:m + 1],
                                             scale=vec_sb[:, 0, m:m + 1])

                # ---- L1: fp8 +-1 DoubleRow matmuls, fused BN1+sign ----
                s1_sb = spool.tile([P, KO, BT], dt.float8e4, tag="s1")
                for m in range(KO):
                    ps = pspool.tile([P, BT], dt.float32, tag="ps")
                    for kp in range(KO // 2):
                        nc.tensor.matmul(ps, w1_sb[:, m, 2 * kp:2 * kp + 2, :],
                                         s0_sb[:, 2 * kp:2 * kp + 2, :],
                                         start=kp == 0, stop=kp == KO // 2 - 1,
                                         perf_mode=DR)
                    nc.scalar.activation(s1_sb[:, m, :], ps, Sign,
                                         bias=vec_sb[:, 3, m:m + 1],
                                         scale=vec_sb[:, 2, m:m + 1])

                # ---- L2: fp8 +-1 DoubleRow matmuls, fused scale+bias ----
                # The last tile evicts 2/2/2/1/1 through a DEDICATED 5-deep
                # buffer tag: no opool wrap-waits delay the final acts, and
                # the final transfer (the kernel tail) is only 256KB.
                if bt < NBT - 1:
                    groups = [(0, 4), (4, 4)]
                else:
                    groups = [(0, 2), (2, 2), (4, 2), (6, 1), (7, 1)]
                for m0, ng in groups:
                    if bt < NBT - 1:
                        o_sb = opool.tile([P, ng, BT], dt.float32, tag="om")
                    else:
                        o_sb = opool.tile([P, ng, BT], dt.float32, tag="omz",
                                          bufs=5)
                    for mi in range(ng):
                        m = m0 + mi
                        ps = pspool.tile([P, BT], dt.float32, tag="ps")
                        for kp in range(KO // 2):
                            nc.tensor.matmul(ps, w2_sb[:, m, 2 * kp:2 * kp + 2, :],
                                             s1_sb[:, 2 * kp:2 * kp + 2, :],
                                             start=kp == 0, stop=kp == KO // 2 - 1,
                                             perf_mode=DR)
                        nc.scalar.activation(o_sb[:, mi, :], ps, Ident,
                                             bias=vec_sb[:, 5, m:m + 1],
                                             scale=vec_sb[:, 4, m:m + 1])
                    nc.sync.dma_start(out_t[:, bt, m0:m0 + ng, :], o_sb)

    nc.compile()
    return nc


def kernel(**inputs) -> np.ndarray:
    global LAST_RESULTS
    f32 = np.float32
    x = np.asarray(inputs["x"], f32)
    W0 = np.asarray(inputs["W0"], f32)
    b0 = np.asarray(inputs["b0"], f32)
    W1 = np.asarray(inputs["W1"], f32)
    b1 = np.asarray(inputs["b1"], f32)
    W2 = np.asarray(inputs["W2"], f32)
    b2 = np.asarray(inputs["b2"], f32)
    bn0_g = np.asarray(inputs["bn0_g"], f32)
    bn0_b = np.asarray(inputs["bn0_b"], f32)
    bn0_rm = np.asarray(inputs["bn0_rm"], f32)
    bn0_rv = np.asarray(inputs["bn0_rv"], f32)
    bn1_g = np.asarray(inputs["bn1_g"], f32)
    bn1_b = np.asarray(inputs["bn1_b"], f32)
    bn1_rm = np.asarray(inputs["bn1_rm"], f32)
    bn1_rv = np.asarray(inputs["bn1_rv"], f32)
    osc = np.asarray(inputs["out_scale"], f32)

    # per-channel affine folds (BN in eval mode):
    #   bn0(h+b0) = h*A0 + B0 ; bn1(h+b1) = h*A1 + B1 ; out = h*CS + CB
    # L0's psum carries h*2^28, so A0 absorbs the 2^-28.
    inv0 = (bn0_g / np.sqrt(bn0_rv + EPS)).astype(f32)
    inv1 = (bn1_g / np.sqrt(bn1_rv + EPS)).astype(f32)
    A0, B0 = (inv0 / PSC).astype(f32), ((b0 - bn0_rm) * inv0 + bn0_b).astype(f32)
    A1, B1 = inv1, ((b1 - bn1_rm) * inv1 + bn1_b).astype(f32)
    CS, CB = osc, (b2 * osc).astype(f32)
    vec = np.stack([A0, B0, A1, B1, CS, CB])           # [6, D]
    vec_host = np.ascontiguousarray(
        vec.reshape(6, KO, P).transpose(2, 0, 1))      # [P, 6, KO]

    def wfmt(a):
        # [D_out, D_in] -> [P(p), KO(m), KO(k), P(c)] with
        # elem[p, m, k, c] = a[m*128 + c, k*128 + p]
        return a.reshape(KO, P, KO, P).transpose(3, 0, 2, 1)

    def xfmt(a):
        # [B, D] -> [P(p), B/BT(bt), KO(k), BT(c)] with
        # elem[p, bt, k, c] = a[bt*BT + c, k*128 + p]
        return a.reshape(-1, BT, KO, P).transpose(3, 0, 2, 1)

    e4 = mybir.dt.np(dt.float8e4)
    W11s = np.float16(W0 * WSC)                        # fp16 main weights
    Wr = W0 - W11s.astype(f32) / WSC
    w0h_host = np.ascontiguousarray(wfmt(W11s))
    wq_host = np.ascontiguousarray(np.concatenate([
        wfmt((W0 * np.float32(2.0**12)).astype(e4)),
        wfmt((Wr * np.float32(2.0**23)).astype(e4)),
    ], axis=2))                                        # [P, KO, 2KO, P]
    w1_host = np.ascontiguousarray(wfmt(np.sign(W1).astype(e4)))
    w2_host = np.ascontiguousarray(wfmt(np.sign(W2).astype(e4)))

    xh11s = np.float16(x * XSC)                        # fp16 main moving
    xl = x - xh11s.astype(f32) / XSC
    xhT = xfmt(xh11s)                                  # [P, B/BT, KO, BT]
    xqT = np.concatenate([
        xfmt((xl * np.float32(2.0**16)).astype(e4)),
        xfmt((x * np.float32(2.0**5)).astype(e4)),
    ], axis=2)                                         # [P, B/BT, 2KO, BT]
    # pack fp8 slots + fp16 main bytes into one dram tensor per batch-tile
    xq_bytes = np.ascontiguousarray(xqT).view(np.uint8).reshape(P, B // BT, XQB)
    xh_bytes = np.ascontiguousarray(xhT).view(np.uint8).reshape(P, B // BT, XHB)

    shared = {
        "w0h": w0h_host, "wq": wq_host,
        "w1": w1_host, "w2": w2_host, "vec": vec_host,
    }
    in_maps = []
    for c in range(NCORES):
        bs = slice(c * NBT, (c + 1) * NBT)
        xin_host = np.concatenate(
            [xq_bytes[:, bs], xh_bytes[:, bs]], axis=2)
        in_maps.append({
            **shared,
            "xin": np.ascontiguousarray(xin_host),
        })

    global _NC
    if _NC is None:
        _NC = _build()
    res = run_bass_kernel_spmd(_NC, in_maps, core_ids=list(range(NCORES)))
    LAST_RESULTS = res

    out = np.empty((B, D), f32)
    for c in range(NCORES):
        # [P, NBT, KO, BT] -> [BS, KO*P] with channel = ko*P + p
        o = res.results[c]["out"].transpose(1, 3, 2, 0).reshape(BS, D)
        out[c * BS:(c + 1) * BS] = o
    return out
